# revision 21
# baseline (speedup 1.0000x reference)
"""GCNConvNet on 8 Trainium2 NeuronCores (Bass/Tile SPMD kernel), v2.

Strategy (graph/data parallel, per sharding hint):
  - Nodes relabeled on host and sharded across 8 cores (degree-snake
    balance).  Each core owns a contiguous range of new ids and computes
    the conv stack for exactly those destination nodes.
  - Per layer: Z = H_shard @ W (bf16) is emitted per dst tile into an
    SBUF-resident z table (zsres, double-buffered by layer parity) and
    written to HBM (zbufQ quarters).  Row-quarter AllGathers start as
    soon as each quarter's z is complete, overlapping the remaining
    compute of the layer.
  - Sparse aggregation sum_{e: dst in shard} norm_e * Z[src_e] via
    (a) dma_gather of Z rows from the AllGathered table (zfullQ) in
        edge-slot order, one 128-slot block per (dst-group, src-quarter)
    (b) one PE matmul per block: psum[feat, dst] += M_block^T @ S_block
        (S holds the GCN edge norms, host-precomputed).
  - The GpSimd Q7 descriptor generation (~2.1 ns/idx serial in the
    dma_gather ucode) is the kernel's hard bottleneck (~1.9 ms busy);
    the schedule keeps it saturated: gathers are consolidated per
    (tile-pair, chunk) and emitted two groups ahead of consumption, and
    each layer boundary hoists ~55 us of chunk-0..2 descriptor work
    ahead of the first chunk-3 gather so the quarter-3 AllGather
    latency is hidden.  (A prepare_only+trigger_dma variant was tried:
    the per-prep InstIncSwdgeSem tax (+1.35 us each on GpSimd) cost
    more than the stalls it removed, and its deferred-write WAR
    tracking raced on m-tile reuse.)
  - Self-loop contribution streams from the SBUF-resident zsres (no HBM
    round trip).  S / Sself are streamed from HBM per tile on the
    Activation DMA queue, freeing SBUF for the gather lookahead.
  - single_packet=True crashes the runtime (NRT INTERNAL); keep False.

kernel(**inputs) takes the FULL inputs and returns the FULL [N,1] output.
"""

import math
import numpy as np
import ml_dtypes

import concourse.bass as bass
import concourse.bacc as bacc
import concourse.tile as tile
import concourse.mybir as mybir
from concourse.bass_utils import run_bass_kernel_spmd

F32 = mybir.dt.float32
BF16 = mybir.dt.bfloat16
I16 = mybir.dt.int16
AF = mybir.ActivationFunctionType
NP_BF16 = ml_dtypes.bfloat16

CORES = 8
CHUNKS = 4  # == row-quarters of the z table
P = 128     # feature dim == partition dim
SINGLE_PACKET = False
TPG = 2     # tiles per gather instruction (consolidation)
LOOKAHEAD = 2  # prep groups ahead of consumption


class Cfg:
    def __init__(self, n_nodes, g=30, tgp=17):
        assert n_nodes % (CORES * CHUNKS) == 0
        self.N = n_nodes
        self.NPC = n_nodes // CORES          # real nodes per core
        self.G = g                           # dsts per group (psum span)
        self.TGP = tgp                       # groups per tile
        self.DT = g * tgp                    # dsts per tile (<=512 psum bank)
        assert self.DT <= 512
        # groups padded so the local id space splits into 4 equal quarters
        self.NGROUPS = ((math.ceil(self.NPC / g) + CHUNKS - 1)
                        // CHUNKS) * CHUNKS
        self.QG = self.NGROUPS // CHUNKS     # groups per quarter
        self.NPCP = self.NGROUPS * g         # padded local id space
        self.QROWS = self.NPCP // CHUNKS     # local rows per quarter
        self.NQR = self.QROWS * CORES        # rows per zfullQ chunk
        assert self.NQR <= 32767             # int16 gather index limit
        self.NP = self.NPCP * CORES
        self.NPCQ = self.NPC // CHUNKS       # real nodes per (core, quarter)
        self.T = math.ceil(self.NGROUPS / tgp)
        self.tile_groups = [
            min(tgp, self.NGROUPS - t * tgp) for t in range(self.T)
        ]
        self.tile_dsts = [ng * g for ng in self.tile_groups]
        self.scols = [CHUNKS * ng * g for ng in self.tile_groups]
        self.s_total = sum(self.scols)
        self.s_base = np.concatenate([[0], np.cumsum(self.scols)[:-1]])
        self.smax = max(self.scols)
        # self-loop diagonal blocks (streamed from zsres)
        self.self_blocks = [math.ceil(dt / 128) for dt in self.tile_dsts]
        self.sself_cols = [nb * 128 for nb in self.self_blocks]
        self.sself_total = sum(self.sself_cols)
        self.ss_base = np.concatenate([[0], np.cumsum(self.sself_cols)[:-1]])
        self.ssmax = max(self.sself_cols)
        # zsres block column base per tile (feat-cols, 128 per block)
        self.z_base = np.concatenate(
            [[0], np.cumsum([nb * 128 for nb in self.self_blocks])[:-1]])
        self.zres_cols = int(sum(nb * 128 for nb in self.self_blocks))
        # --- tile groups-of-TPG (gather consolidation) ---
        self.pairs = [tuple(range(TPG * i, min(TPG * (i + 1), self.T)))
                      for i in range(math.ceil(self.T / TPG))]
        self.NGRP = len(self.pairs)
        self.grp_of = [t // TPG for t in range(self.T)]
        self.pb = [sum(self.tile_groups[t] for t in pr) for pr in self.pairs]
        self.MAXB = max(self.pb)
        # idx cols per (pair, chunk) = pb*128/16 = pb*8
        self.pidx_base = np.concatenate(
            [[0], np.cumsum([CHUNKS * b * 8 for b in self.pb])[:-1]])
        self.idx_total = int(sum(CHUNKS * b * 8 for b in self.pb))
        # block offset of tile within its group
        self.boff = [sum(self.tile_groups[u] for u in
                         range(TPG * (t // TPG), t))
                     for t in range(self.T)]
        # first tile whose z-emit completes quarter c
        self.q_done_tile = [
            math.ceil(self.QG * (c + 1) / tgp) - 1 for c in range(CHUNKS)
        ]
        # quarters 0..2 trigger their AG as soon as complete (+2 tiles slack
        # so the AG's input wait doesn't head-of-line-block the gpsimd queue)
        self.trig_tile = [
            min(t + 2, self.T - 1) for t in self.q_done_tile[:CHUNKS - 1]
        ]


# ---------------------------------------------------------------------------
# host preprocessing
# ---------------------------------------------------------------------------

def _group_greedy(dvec, n_groups, gsize, cap=128):
    """Assign len(dvec) items into n_groups of <=gsize items each,
    keeping every per-chunk (4-dim) load <= cap.  dvec: [n,4] int."""
    n = dvec.shape[0]
    order = np.argsort(-dvec.sum(axis=1), kind="stable")
    loads = np.zeros((n_groups, CHUNKS), np.int64)
    sizes = np.zeros(n_groups, np.int64)
    group_of = np.empty(n, np.int64)
    for it, i in enumerate(order):
        cand = np.max(loads + dvec[i], axis=1).astype(np.float64)
        cand[sizes >= gsize] = np.inf
        # keep group sizes growing in lockstep so late (small) items always
        # have several candidate groups to choose from
        lim = sizes.min() + 2
        cand[sizes >= lim] = np.inf
        g = int(np.argmin(cand))
        group_of[i] = g
        loads[g] += dvec[i]
        sizes[g] += 1
    # repair pass: improving swaps until all chunk loads <= cap
    stall = 0
    for _ in range(60000):
        gbad, cbad = np.unravel_index(np.argmax(loads), loads.shape)
        worst = loads[gbad, cbad]
        if worst <= cap or stall > 40:
            break
        members = np.flatnonzero(group_of == gbad)
        others = np.flatnonzero(group_of != gbad)
        do = dvec[others]
        improved = False
        for i in members[np.argsort(-dvec[members, cbad])[:4]]:
            di = dvec[i]
            base_g = loads[gbad] - di
            cand_g = np.max(base_g + do, axis=1)
            base_o = loads[group_of[others]] - do
            cand_o = np.max(base_o + di, axis=1)
            score = np.maximum(cand_g, cand_o)
            j = others[int(np.argmin(score))]
            if score.min() < worst:
                gj = group_of[j]
                loads[gbad] += dvec[j] - di
                loads[gj] += di - dvec[j]
                group_of[i] = gj
                group_of[j] = gbad
                improved = True
                break
        stall = 0 if improved else stall + 1
    return group_of, loads


def preprocess(x, edge_index, cfg: Cfg):
    N = cfg.N
    src_o = np.asarray(edge_index[0], np.int64)
    dst_o = np.asarray(edge_index[1], np.int64)

    deg = np.bincount(dst_o, minlength=N).astype(np.float64) + 1.0
    dinv = (1.0 / np.sqrt(deg)).astype(np.float64)

    srcs = src_o
    dsts = dst_o
    norms = (dinv[srcs] * dinv[dsts]).astype(np.float32)
    norm_self = (dinv * dinv).astype(np.float32)

    # --- core assignment: snake over degree-sorted nodes ---
    order = np.argsort(-deg, kind="stable")
    pattern = np.concatenate([np.arange(CORES), np.arange(CORES)[::-1]])
    reps = math.ceil(N / (2 * CORES))
    core_seq = np.tile(pattern, reps)[:N]
    core_of = np.empty(N, np.int64)
    core_of[order] = core_seq
    counts = np.bincount(core_of, minlength=CORES)
    assert (counts == cfg.NPC).all(), counts

    # --- quarter assignment within each core (snake by degree again) ---
    quarter_of = np.empty(N, np.int64)
    qpat = np.concatenate([np.arange(CHUNKS), np.arange(CHUNKS)[::-1]])
    for k in range(CORES):
        nodes_k = np.flatnonzero(core_of == k)
        order_k = np.argsort(-deg[nodes_k], kind="stable")
        seq = np.tile(qpat, math.ceil(len(nodes_k) / (2 * CHUNKS)))
        quarter_of[nodes_k[order_k]] = seq[:len(nodes_k)]
    for k in range(CORES):
        qc = np.bincount(quarter_of[core_of == k], minlength=CHUNKS)
        assert (qc == cfg.NPCQ).all(), qc

    # --- per-dst chunk-degree vectors (chunk == src quarter) ---
    dvec = np.zeros((N, CHUNKS), np.int64)
    np.add.at(dvec, (dsts, quarter_of[srcs]), 1)

    # --- within-(core, quarter) grouping (4-dim balanced) ---
    local_of = np.empty(N, np.int64)
    for k in range(CORES):
        for q in range(CHUNKS):
            nodes_kq = np.flatnonzero((core_of == k) & (quarter_of == q))
            gof, loads = _group_greedy(dvec[nodes_kq], cfg.QG, cfg.G)
            assert loads.max() <= 128, (
                f"group chunk load {loads.max()} exceeds 128; lower cfg.G"
            )
            o = np.argsort(gof, kind="stable")
            gsorted = gof[o]
            first = np.r_[0, np.flatnonzero(np.diff(gsorted)) + 1]
            fo = np.zeros(cfg.QG, np.int64)
            fo[gsorted[first]] = first
            rank_in_group = np.arange(len(o)) - fo[gsorted]
            local_of[nodes_kq[o]] = (
                (q * cfg.QG + gsorted) * cfg.G + rank_in_group)
    new_of = core_of * cfg.NPCP + local_of
    newpos_of_old = new_of.copy()

    # --- edge bucket packing ---
    e_dst = new_of[dsts]
    e_src = new_of[srcs]
    e_core = e_dst // cfg.NPCP
    e_ldst = e_dst % cfg.NPCP
    e_g = e_ldst // cfg.G            # global group within core
    e_pos = e_ldst % cfg.G
    e_t = e_g // cfg.TGP
    e_gt = e_g % cfg.TGP
    e_srcl = e_src % cfg.NPCP
    e_chunk = e_srcl // cfg.QROWS    # src quarter
    e_lsrc = (e_src // cfg.NPCP) * cfg.QROWS + (e_srcl % cfg.QROWS)

    bucket = ((e_core * cfg.NGROUPS + e_g) * CHUNKS) + e_chunk
    # secondary sort by src row: ascending HBM addresses within a bucket
    so = np.lexsort((e_lsrc, bucket))
    sb = bucket[so]
    starts = np.r_[0, np.flatnonzero(np.diff(sb)) + 1]
    uniq = sb[starts]
    sizes = np.diff(np.r_[starts, len(sb)])
    assert sizes.max() <= 128, f"bucket overflow: {sizes.max()}"
    start_of = np.zeros(cfg.NGROUPS * CORES * CHUNKS, np.int64)
    start_of[uniq] = starts
    rank = np.arange(len(sb)) - start_of[sb]

    # idx / S arrays.  Padding slots point at valid rows (S value 0),
    # spread across the chunk to avoid hot-spotting an HBM row.
    rng_pad = np.random.default_rng(12345)
    idx_all = rng_pad.integers(0, cfg.NQR, (CORES, 16, cfg.idx_total),
                               dtype=np.int16)
    s_all = np.zeros((CORES, P, cfg.s_total), NP_BF16)

    r = rank  # slot-in-block for each sorted edge
    core_s = e_core[so]
    t_s = e_t[so]
    gt_s = e_gt[so]
    c_s = e_chunk[so]
    pos_s = e_pos[so]
    lsrc_s = e_lsrc[so]
    norm_s = norms[so]

    # pair-consolidated idx layout: per pair g: [c][tile-in-pair][gt][slot]
    pt = np.array(cfg.grp_of)[t_s]      # pair index
    boff_arr = np.array(cfg.boff)[t_s]  # block offset of tile within pair
    pb_arr = np.array(cfg.pb)[pt]
    slot = (boff_arr + gt_s) * 128 + r
    icol = np.array(cfg.pidx_base)[pt] + c_s * pb_arr * 8 + (slot // 16)
    irow = slot % 16
    idx_all[core_s, irow, icol] = lsrc_s.astype(np.int16)

    ngt = np.array(cfg.tile_groups)[t_s]
    scol = np.array(cfg.s_base)[t_s] + (c_s * ngt + gt_s) * cfg.G + pos_s
    s_all[core_s, r, scol] = norm_s.astype(NP_BF16)

    # S_self: per-tile diagonal blocks scaling the core's own z rows
    sself_all = np.zeros((CORES, P, cfg.sself_total), NP_BF16)
    dtile = cfg.G * cfg.TGP
    n_core = new_of // cfg.NPCP
    n_local = new_of % cfg.NPCP
    n_t = (n_local // cfg.G) // cfg.TGP
    n_pos = n_local - n_t * dtile
    sself_all[n_core, n_pos % 128, np.array(cfg.ss_base)[n_t] + n_pos] = (
        norm_self.astype(NP_BF16))

    # x shards, feature-major, zero-padded at hole ids
    xT_all = np.zeros((CORES, P, cfg.NPCP), NP_BF16)
    xT_all[n_core, :, n_local] = np.asarray(x, np.float32).astype(NP_BF16)

    return dict(idx_all=idx_all, s_all=s_all, sself_all=sself_all,
                xT_all=xT_all, newpos_of_old=newpos_of_old)


# ---------------------------------------------------------------------------
# bass kernel
# ---------------------------------------------------------------------------

def build_nc(cfg: Cfg):
    nc = bacc.Bacc("TRN2", target_bir_lowering=False, debug=False,
                   num_devices=CORES, num_swdge_queues=4)

    xT = nc.dram_tensor("xT", [P, cfg.NPCP], BF16, kind="ExternalInput")
    idxd = nc.dram_tensor("idx", [16, cfg.idx_total], I16,
                          kind="ExternalInput")
    sd = nc.dram_tensor("S", [P, cfg.s_total], BF16, kind="ExternalInput")
    ssd = nc.dram_tensor("Sself", [P, cfg.sself_total], BF16,
                         kind="ExternalInput")
    wd = nc.dram_tensor("W", [P, 4 * P], BF16, kind="ExternalInput")
    bd = nc.dram_tensor("B", [P, 4], F32, kind="ExternalInput")
    lw1d = nc.dram_tensor("lw1", [P, 64], BF16, kind="ExternalInput")
    lb1d = nc.dram_tensor("lb1", [64, 1], F32, kind="ExternalInput")
    lw2d = nc.dram_tensor("lw2", [64, 1], BF16, kind="ExternalInput")
    lb2d = nc.dram_tensor("lb2", [1, 1], F32, kind="ExternalInput")
    outd = nc.dram_tensor("out", [cfg.NPCP, 1], F32, kind="ExternalOutput")

    # local z rows, one tensor per quarter (AG-input granularity)
    zbufQ = [nc.dram_tensor(f"zbufQ{c}", [cfg.QROWS, P], BF16)
             for c in range(CHUNKS)]
    # AllGather outputs, double-buffered by z-generation parity
    zfullQ = [[nc.dram_tensor(f"zfullQ{p}_{c}", [cfg.NQR, P], BF16,
                              addr_space="Shared")
               for c in range(CHUNKS)] for p in range(2)]

    with tile.TileContext(nc) as tc:
        with tc.tile_pool(name="const", bufs=1) as cp, \
             tc.tile_pool(name="sb", bufs=2) as sbp, \
             tc.tile_pool(name="spool", bufs=3) as sp_s, \
             tc.tile_pool(name="mpool", bufs=12) as mp, \
             tc.tile_pool(name="psagg", bufs=3, space="PSUM") as pp_agg, \
             tc.tile_pool(name="psz", bufs=2, space="PSUM") as pp_z, \
             tc.tile_pool(name="pshead", bufs=1, space="PSUM") as pp_head:

            # gather indices first: layer-0 preps need only these
            idx_sb = cp.tile([P, cfg.idx_total], I16)
            for q in range(8):
                nc.scalar.dma_start(idx_sb[16 * q:16 * (q + 1), :], idxd[:, :])
            w_sb = cp.tile([P, 4 * P], BF16)
            nc.scalar.dma_start(w_sb[:], wd[:, :])
            b_sb = cp.tile([P, 4], F32)
            nc.scalar.dma_start(b_sb[:], bd[:, :])
            lw1_sb = cp.tile([P, 64], BF16)
            nc.scalar.dma_start(lw1_sb[:], lw1d[:, :])
            lb1_sb = cp.tile([64, 1], F32)
            nc.scalar.dma_start(lb1_sb[:], lb1d[:, :])
            lw2_sb = cp.tile([64, 1], BF16)
            nc.scalar.dma_start(lw2_sb[:], lw2d[:, :])
            lb2_sb = cp.tile([1, 1], F32)
            nc.scalar.dma_start(lb2_sb[:], lb2d[:, :])
            # SBUF-resident z table, double-buffered by parity
            zsres = [cp.tile([P, cfg.zres_cols], BF16, name=f"zsres{p}")
                     for p in range(2)]

            # ---- helpers -------------------------------------------------
            def z_write(par, t, b, rows):
                """DMA zsres[par] block (t,b) -> zbufQ rows, split at
                quarter boundaries; alternate issue queues to halve the
                per-queue sequencer config serialization."""
                eng = nc.sync if b % 2 == 0 else nc.scalar
                c0 = int(cfg.z_base[t]) + b * 128
                a, off, rem = t * cfg.DT + b * 128, 0, rows
                while rem > 0:
                    q, qa = divmod(a, cfg.QROWS)
                    n = min(rem, cfg.QROWS - qa)
                    eng.dma_start(
                        zbufQ[q][qa:qa + n, :],
                        zsres[par][off:off + n, c0:c0 + P])
                    a += n
                    off += n
                    rem -= n

            def emit_z(h_tile, layer, t):
                """z rows for tile t of layer `layer` (reads W[layer]),
                into zsres[layer%2] and zbufQ."""
                par = layer % 2
                dt = cfg.tile_dsts[t]
                for b in range(cfg.self_blocks[t]):
                    s0 = b * 128
                    sl = min(P, dt - s0)
                    zp = pp_z.tile([P, P], F32, tag="zp",
                                   name=f"zp{layer}_{t}_{b}")
                    nc.tensor.matmul(
                        zp[0:sl, :],
                        lhsT=h_tile[:, s0:s0 + sl],
                        rhs=w_sb[:, layer * P:(layer + 1) * P],
                        start=True, stop=True)
                    c0 = int(cfg.z_base[t]) + s0
                    nc.vector.tensor_copy(
                        zsres[par][0:sl, c0:c0 + P], zp[0:sl, :])
                    z_write(par, t, b, sl)

            def ag(gen, c):
                nc.gpsimd.collective_compute(
                    "AllGather", mybir.AluOpType.bypass,
                    replica_groups=[list(range(CORES))],
                    ins=[zbufQ[c].ap()], outs=[zfullQ[gen % 2][c].ap()])

            def emit_head(h_tile, t):
                dt = cfg.tile_dsts[t]
                r0 = t * cfg.DT
                hp = pp_head.tile([64, cfg.DT], F32, tag="hp", name=f"hp{t}")
                nc.tensor.matmul(hp[:, 0:dt], lhsT=lw1_sb[:],
                                 rhs=h_tile[:, 0:dt], start=True, stop=True)
                ha = sbp.tile([64, cfg.DT], BF16, tag="ha", name=f"ha{t}")
                nc.scalar.activation(ha[:, 0:dt], hp[:, 0:dt], AF.Relu,
                                     bias=lb1_sb[:])
                op = pp_head.tile([1, cfg.DT], F32, tag="op", name=f"op{t}")
                nc.tensor.matmul(op[:, 0:dt], lhsT=lw2_sb[:],
                                 rhs=ha[0:64, 0:dt], start=True, stop=True)
                ob = sbp.tile([1, cfg.DT], F32, tag="ob", name=f"ob{t}")
                nc.scalar.activation(ob[:, 0:dt], op[:, 0:dt], AF.Sigmoid,
                                     bias=lb2_sb[:])
                nc.sync.dma_start(
                    outd[r0:r0 + dt, :].rearrange("a b -> b a"), ob[:, 0:dt])

            mcache = {}

            def gather_group(layer, g, par, chunks):
                """Gathers for tile-pair g, the given chunks."""
                nb = cfg.pb[g]
                slots = nb * 128
                for c in chunks:
                    if (g, c) in mcache:
                        continue
                    m = mp.tile([P, cfg.MAXB * P], BF16, tag="m",
                                name=f"m{layer}_{g}_{c}")
                    m3 = m[:, 0:nb * P].rearrange("p (b e) -> p b e", e=P)
                    ic0 = int(cfg.pidx_base[g]) + c * nb * 8
                    nc.gpsimd.dma_gather(
                        m3, zfullQ[par][c][:, :],
                        idx_sb[:, ic0:ic0 + nb * 8],
                        slots, slots, P, single_packet=SINGLE_PACKET,
                        queue_num=c)
                    mcache[(g, c)] = m

            def load_s(t):
                st = sp_s.tile([P, cfg.smax], BF16, tag="S", name=f"S{t}")
                nc.scalar.dma_start(
                    st[:, 0:cfg.scols[t]],
                    sd[:, int(cfg.s_base[t]):int(cfg.s_base[t])
                       + cfg.scols[t]])
                sst = sp_s.tile([P, cfg.ssmax], BF16, tag="SS",
                                name=f"SS{t}")
                nc.scalar.dma_start(
                    sst[:, 0:cfg.sself_cols[t]],
                    ssd[:, int(cfg.ss_base[t]):int(cfg.ss_base[t])
                        + cfg.sself_cols[t]])
                return st, sst

            scache = {}

            def consume_tile(layer, t):
                """Aggregation + activation (+ next-layer z or head)."""
                par = layer % 2
                g = cfg.grp_of[t]
                ng = cfg.tile_groups[t]
                dt = cfg.tile_dsts[t]
                st, sst = scache.pop(t)
                ps = pp_agg.tile([P, cfg.DT], F32, tag="agg",
                                 name=f"agg{layer}_{t}")
                k = 0
                for c in range(CHUNKS):
                    m = mcache[(g, c)]
                    for gt in range(ng):
                        bcol = (cfg.boff[t] + gt) * P
                        nc.tensor.matmul(
                            ps[:, gt * cfg.G:(gt + 1) * cfg.G],
                            lhsT=m[:, bcol:bcol + P],
                            rhs=st[:, (c * ng + gt) * cfg.G:
                                   (c * ng + gt + 1) * cfg.G],
                            start=(k == 0), stop=False)
                        k += 1
                # self-loop: own z rows (SBUF-resident) * diag(norm_self)
                nsb = cfg.self_blocks[t]
                for b in range(nsb):
                    rows = min(P, dt - b * P)
                    zc0 = int(cfg.z_base[t]) + b * 128
                    nc.tensor.matmul(
                        ps[:, b * P:b * P + rows],
                        lhsT=zsres[par][0:rows, zc0:zc0 + P],
                        rhs=sst[0:rows, b * P:b * P + rows],
                        start=False, stop=(b == nsb - 1))
                h = sbp.tile([P, cfg.DT], BF16, tag="h",
                             name=f"h{layer}_{t}")
                nc.scalar.activation(
                    h[:, 0:dt], ps[:, 0:dt],
                    AF.Relu if layer < 3 else AF.Identity,
                    bias=b_sb[:, layer:layer + 1])
                if layer < 3:
                    emit_z(h, layer + 1, t)
                    for c in range(CHUNKS - 1):
                        if cfg.trig_tile[c] == t:
                            ag(layer + 1, c)
                else:
                    emit_head(h, t)

            def boundary(layer, par):
                """Hoisted gathers covering the AG waits: chunk-0/1
                descriptor generation precedes the chunk-2 gathers
                (AG-2 triggers late in the previous layer), and ~55us
                of work precedes the first chunk-3 gather."""
                gather_group(layer, 0, par, [0])
                gather_group(layer, 1, par, [0])
                if cfg.NGRP > 2:
                    gather_group(layer, 2, par, [0])
                gather_group(layer, 0, par, [1])
                gather_group(layer, 1, par, [1])
                ag(layer, CHUNKS - 1)
                gather_group(layer, 0, par, [2])
                gather_group(layer, 1, par, [2])
                gather_group(layer, 0, par, [CHUNKS - 1])

            # ---- prologue: layer-0 z from x ------------------------------
            for t in range(cfg.T):
                dt = cfg.tile_dsts[t]
                r0 = t * cfg.DT
                xt = sbp.tile([P, cfg.DT], BF16, tag="xt", name=f"xt{t}")
                nc.scalar.dma_start(xt[:, 0:dt], xT[:, r0:r0 + dt])
                emit_z(xt, 0, t)
                for c in range(CHUNKS - 1):
                    if cfg.q_done_tile[c] == t:
                        ag(0, c)
            for gg in range(LOOKAHEAD):      # S for the first groups
                for t in cfg.pairs[gg]:
                    scache[t] = load_s(t)
            boundary(0, 0)

            # ---- conv layers --------------------------------------------
            for layer in range(4):
                par = layer % 2
                for g in range(cfg.NGRP):
                    if g + LOOKAHEAD < cfg.NGRP:
                        gather_group(layer, g + LOOKAHEAD, par,
                                     range(CHUNKS - 1))
                    if g + 1 < cfg.NGRP:
                        gather_group(layer, g + 1, par, [CHUNKS - 1])
                    for t in cfg.pairs[g]:
                        if t == cfg.pairs[g][0] and g + 1 < cfg.NGRP:
                            for tn in cfg.pairs[g + 1]:  # S one grp ahead
                                if tn not in scache:
                                    scache[tn] = load_s(tn)
                        consume_tile(layer, t)
                    # free consumed m tiles
                    for c in range(CHUNKS):
                        mcache.pop((g, c), None)
                if layer < 3:
                    boundary(layer + 1, 1 - par)
                    for gg in range(LOOKAHEAD):
                        for t in cfg.pairs[gg]:
                            scache[t] = load_s(t)

    nc.compile()
    return nc


# ---------------------------------------------------------------------------
# entry point
# ---------------------------------------------------------------------------

_CACHE = {}


def _get_nc(cfg: Cfg):
    key = (cfg.N, cfg.G, cfg.TGP)
    if key not in _CACHE:
        _CACHE[key] = build_nc(cfg)
    return _CACHE[key]


def run(x, edge_index, w0, b0, w1, b1, w2, b2, w3, b3, lw1, lb1, lw2, lb2,
        cfg: Cfg):
    pre = preprocess(x, edge_index, cfg)
    W = np.concatenate([np.asarray(w, np.float32)
                        for w in (w0, w1, w2, w3)], axis=1).astype(NP_BF16)
    B = np.stack([np.asarray(b, np.float32)
                  for b in (b0, b1, b2, b3)], axis=1)        # [128, 4] f32
    in_maps = []
    for k in range(CORES):
        in_maps.append({
            "xT": pre["xT_all"][k],
            "idx": pre["idx_all"][k],
            "S": pre["s_all"][k],
            "Sself": pre["sself_all"][k],
            "W": W,
            "B": B,
            "lw1": np.asarray(lw1, np.float32).astype(NP_BF16),
            "lb1": np.asarray(lb1, np.float32).reshape(64, 1),
            "lw2": np.asarray(lw2, np.float32).astype(NP_BF16),
            "lb2": np.asarray(lb2, np.float32).reshape(1, 1),
        })
    nc = _get_nc(cfg)
    res = run_bass_kernel_spmd(nc, in_maps, core_ids=list(range(CORES)))
    out_new = np.concatenate([res.results[k]["out"] for k in range(CORES)],
                             axis=0)  # [NP, 1] in padded new-id order
    out = out_new[pre["newpos_of_old"]]
    return out, res


def make_cfg(n_nodes):
    return Cfg(n_nodes, g=30, tgp=17)


def kernel(x, edge_index, batch, w0, b0, w1, b1, w2, b2, w3, b3,
           lw1, lb1, lw2, lb2):
    x = np.asarray(x, np.float32)
    cfg = make_cfg(x.shape[0])
    out, _ = run(x, edge_index, w0, b0, w1, b1, w2, b2, w3, b3,
                 lw1, lb1, lw2, lb2, cfg)
    return out
